# revision 33
# baseline (speedup 1.0000x reference)
"""Trainium2 Bass kernel for nn_NewDAGExecutor (plan-predictor matmul + 8-step DAG).

Strategy (8 NeuronCores, data-parallel over the 16384 tokens, 2048 tokens/core):
  - Host: transpose each core's token shard to [H, tok], split fp32 into an
    exact fp16 hi/lo pair (lo scaled by 2^11), same for the concatenated
    weight matrix W = [W_init; W_op; W_gate] (168 outputs) and biases.
  - Device: plan = hidden @ W.T + b computed as 3 fp16 matmul passes
    (hi*Whi -> PSUM1, hi*Wlo + lo*Whi -> PSUM2, combine P1 + 2^-11*P2),
    which is fp32-accurate (fp16 products are exact in the fp32 MACs; the
    dropped lo*lo term is ~2^-24 relative). Tokens ride the PSUM partition
    axis so the DAG math lands in token-major layout with no transpose.
  - The 8 sequential DAG steps run wide across all 2048 tokens/core
    ([128 partitions x 16 tile-columns]) on DVE/ACT. tanh(x*1e4) is computed
    via exp to keep every per-step ACT call inside the natural_log_exp table
    set (no per-step table switches); the sign is restored with bitwise ops.
"""

import numpy as np

import concourse.bacc as bacc
import concourse.bass as bass
import concourse.tile as tile
import concourse.mybir as mybir
from concourse.bass_utils import run_bass_kernel_spmd

import concourse.dve_ops as _dve_ops_mod
from concourse.dve_ops import DveOp as _DveOp, LN_BWD_DX_ANT
from concourse.dve_spec import (
    Spec as _Spec, Src0 as _S0, Src1 as _S1, C0 as _C0, C1 as _C1, C2 as _C2,
    Zero as _Z, One as _One, lower as _dve_lower, _has_src1, maxx as _maxx,
)
from concourse.dve_uop import DveOpSpec as _DveOpSpec


def _register_dve_op(name, spec, subdim=False):
    """Register a custom DVE op at import time (documented extension point:
    dve_ops.OPS + the name->row map; sha computed here so compile()'s
    drift check is self-consistent)."""
    if name in _dve_ops_mod._SUB_OPCODE_FOR_NAME:
        for op in _dve_ops_mod.OPS:
            if op.name == name:
                return op
    opcode = _dve_ops_mod._CUSTOM_DVE_ROW_BASE + len(_dve_ops_mod.OPS)
    shas = {}
    for ver in ("v3", "v4"):
        s = _DveOpSpec(name=name, opcode=opcode, uops=_dve_lower(spec, ver=ver),
                       rd1_en=_has_src1(spec))
        shas[ver] = s.sha(ver)
    op = _DveOp(name, spec, subdim, shas)
    _dve_ops_mod.OPS.append(op)
    _dve_ops_mod._SUB_OPCODE_FOR_NAME[name] = opcode
    _dve_ops_mod.CUSTOM_DVE_SPECS[name] = spec
    return op


# |x| + 1 (for the sign_prod factors)
ABS1 = _register_dve_op("ANT_ABS1", _Spec(
    body=_maxx(_S0, _Z - _S0) + _One,
    reference=lambda in0, in1, s0, s1, imm2: np.abs(in0.astype(np.float32)) + 1.0,
))

# imm2 * |x| (prescale for exp(-2e4|x|))
ABSM = _register_dve_op("ANT_ABSM", _Spec(
    body=_maxx(_S0, _Z - _S0) * _C2,
    reference=lambda in0, in1, s0, s1, imm2: np.abs(in0.astype(np.float32)) * imm2,
))

# Newton reciprocal of d = 1+e, e in [0,1]: minimax linear seed on [1,2]
# (a=24/17, b=-8/17, seed rel err 1/17) + one Newton pass -> y1.
_d_node = _One + _S0
_y0_node = _d_node * _C1 + _C0
TANH_A = _register_dve_op("ANT_TANH_A", _Spec(
    body=_y0_node * (_C2 - _d_node * _y0_node),
    reference=lambda in0, in1, s0, s1, imm2: (
        lambda d: (lambda y0: y0 * (imm2 - d * y0))(d * s1 + s0))(
            1.0 + in0.astype(np.float32)),
))

# Second Newton pass + (1-e) numerator: out = (1-e) * y1*(2 - (1+e)*y1)
TANH_B = _register_dve_op("ANT_TANH_B", _Spec(
    body=(_One - _S0) * (_S1 * (_C0 - _d_node * _S1)),
    reference=lambda in0, in1, s0, s1, imm2: (
        (1.0 - in0.astype(np.float32)) *
        (in1 * (s0 - (1.0 + in0.astype(np.float32)) * in1))),
))

_TANH_C0 = 24.0 / 17.0
_TANH_C1 = -8.0 / 17.0

# Pin ACT table-set choice: the greedy first-match in insert_act_table_loads
# would alternate natural_log <-> exp_and_others every DAG step (2 x 2.7us per
# step). Advertise ln/exp only in natural_log_exp_and_others and tanh/sigmoid
# only in sigmoid_and_others so the whole kernel needs exactly 2 table loads.
_ORIG_GAT = bacc.get_activation_tables


def _pinned_activation_tables(arch):
    tables = _ORIG_GAT(arch)
    LN = mybir.ActivationFunctionType.Ln
    EXP = mybir.ActivationFunctionType.Exp
    TANH = mybir.ActivationFunctionType.Tanh
    SIG = mybir.ActivationFunctionType.Sigmoid
    for name, funcs in tables.items():
        if name != "natural_log_exp_and_others":
            funcs.discard(LN)
            funcs.discard(EXP)
        if name != "sigmoid_and_others":
            funcs.discard(TANH)
            funcs.discard(SIG)
    return tables


bacc.get_activation_tables = _pinned_activation_tables

F32 = mybir.dt.float32
F16 = mybir.dt.float16
U32 = mybir.dt.uint32
ALU = mybir.AluOpType
ACTF = mybir.ActivationFunctionType
AXX = mybir.AxisListType.X

NCORES = 8
B, T, H = 4, 4096, 2048
NTOK = B * T                    # 16384
TPC = NTOK // NCORES            # 2048 tokens per core
NTILE = TPC // 128              # 16 token tiles per core
KCH = H // 128                  # 16 contraction chunks
NN = 16                         # DAG nodes
INTER = 8                       # steps
INIT_SLOTS = 8
# Pruned plan layout: only outputs the DAG actually reads.
#   [0:8]    init magnitudes for slots 0-7 (slots 8-15 are dead: they are
#            rewritten at step s=j-8 before any step can read them)
#   [8:24]   init signs, all 16 slots (sign_prod reads every slot)
#   [24:116] op rows packed: step s contributes its first 8+s nodes only
#            (the causal mask zeroes the rest)
#   [116:124] gates
OPOFF = [24]
for _s in range(1, INTER):
    OPOFF.append(OPOFF[-1] + INIT_SLOTS + _s - 1)
NF = 124
LOG_CLAMP = 23.026
SCL = 2048.0                    # 2^11 lo-part scale
ISCL = 1.0 / SCL
NCHUNKS = 2                     # DAG token-chunks (overlap DAG with later matmuls)
BATCH_EXP = True                # batch et+er exps into one ACT call per step
SPLIT_R = True                  # prefix/last-col split of the R reductions

_CACHE = {}


def _build(repeats=1, parts="all"):
    nc = bacc.Bacc("TRN2", target_bir_lowering=False, debug=False)

    hf_d = nc.dram_tensor("hf", [NTILE // 2, 128, 2, KCH, 256], F16,
                          kind="ExternalInput")
    wt_d = nc.dram_tensor("wt", [H, 2 * NF], F16, kind="ExternalInput")
    bias_d = nc.dram_tensor("bias", [1, 2 * NF], F16, kind="ExternalInput")
    out_d = nc.dram_tensor("out", [128, NTILE], F32, kind="ExternalOutput")

    with tile.TileContext(nc) as tc:
        with tc.tile_pool(name="consts", bufs=1) as consts, \
             tc.tile_pool(name="hfp", bufs=3) as hfp, \
             tc.tile_pool(name="evp", bufs=3) as evp, \
             tc.tile_pool(name="ns", bufs=2) as ns, \
             tc.tile_pool(name="pp", bufs=3, space="PSUM") as pp:

            wt_sb = consts.tile([128, KCH, 2 * NF], F16)
            nc.sync.dma_start(out=wt_sb, in_=wt_d.rearrange("(k p) f -> p k f", p=128))
            bias_sb = consts.tile([1, 2 * NF], F16)
            nc.sync.dma_start(out=bias_sb, in_=bias_d[:, :])
            ones = consts.tile([1, 128], F16)
            nc.vector.memset(ones, 1.0)

            for _rep in range(repeats):
                _emit_body(nc, tc, consts, hfp, evp, ns, pp,
                           hf_d, wt_sb, bias_sb, ones, out_d, parts=parts)

    nc.compile()
    return nc


def _emit_body(nc, tc, consts, hfp, evp, ns, pp, hf_d, wt_sb, bias_sb, ones, out_d,
               parts="all"):
    cw = NTILE // NCHUNKS
    chunks = [(c * cw, (c + 1) * cw, chr(65 + c)) for c in range(NCHUNKS)]
    plan = {sfx: consts.tile([128, cw, NF], F32, tag=f"plan{sfx}", name=f"plan{sfx}")
            for _, _, sfx in chunks}

    # ---- plan predictor: 3-pass fp16 matmul per token tile ----
    if parts == "dag":
        for _, _, sfx in chunks:
            nc.vector.memset(plan[sfx], 0.5)
    for grp in range(NTILE // 2) if parts != "dag" else []:
        hf_sb = hfp.tile([128, 2, KCH, 256], F16, tag="hf")
        nc.sync.dma_start(out=hf_sb, in_=hf_d[grp])
        for t in range(2):
            i = grp * 2 + t
            lo_t, _, sfx = chunks[i // cw]
            p12 = pp.tile([128, 2 * NF], F32, tag="p12")
            nc.tensor.matmul(p12, ones[:, :], bias_sb[:, :], start=True, stop=False)
            for k in range(KCH):
                hi = hf_sb[:, t, k, 0:128]
                lo = hf_sb[:, t, k, 128:256]
                wh = wt_sb[:, k, 0:NF]
                whl = wt_sb[:, k, :]
                # hi * [Wh | Wl] in one N=248 matmul; lo * Wh into the hi-Wl half
                nc.tensor.matmul(p12, hi, whl, start=False, stop=False)
                nc.tensor.matmul(p12[:, NF:2 * NF], lo, wh, start=False,
                                 stop=(k == KCH - 1), skip_group_check=True)
            tmp = evp.tile([128, NF], F32, tag="ev")
            nc.scalar.activation(tmp, p12[:, NF:2 * NF], ACTF.Copy, bias=0.0,
                                 scale=ISCL)
            nc.vector.tensor_tensor(out=plan[sfx][:, i - lo_t, :],
                                    in0=p12[:, 0:NF], in1=tmp, op=ALU.add)

    if parts == "mm":
        nc.sync.dma_start(out=out_d[:, :], in_=plan[chunks[0][2]][:, 0, 0:NTILE])
        return

    st = {}
    for _, _, sfx in chunks:
        st[sfx] = _dag_init(nc, consts, ns, plan[sfx], sfx, cw)
    for _, _, sfx in chunks:
        _dag_init2(nc, ns, plan[sfx], st[sfx], sfx, cw)
    for s in range(INTER):
        for _, _, sfx in chunks:
            _dag_step(nc, ns, plan[sfx], st[sfx], sfx, cw, s)
    for lo_t, hi_t, sfx in chunks:
        nc.sync.dma_start(out=out_d[:, lo_t:hi_t], in_=st[sfx]["OUT"])


def _flat(t, n, off=0):
    """[P, ...] contiguous tile -> [P, n] flat free view at element offset."""
    return bass.AP(tensor=t.tensor, offset=t.offset + off, ap=[t.ap[0], [1, n]])


def _dag_init(nc, consts, ns, PLAN, sfx, cw):
    st = {}
    G = st["G"] = consts.tile([128, cw, INTER], F32, tag=f"G{sfx}", name=f"G{sfx}")
    VSIGN = st["VSIGN"] = consts.tile([128, cw, NN], F32, tag=f"VSIGN{sfx}", name=f"VSIGN{sfx}")
    VMAG = consts.tile([128, cw, INIT_SLOTS], F32, tag=f"VMAG{sfx}")
    # LMD rows: 0 = DIFF (signed - log), 1 = LOGMAG — diff first so the
    # m12 reduce emits (r2, r1) pairs ready for the R FMA-scan.
    LMD = st["LMD"] = consts.tile([128, 2, cw, NN], F32, tag=f"LMD{sfx}", name=f"LMD{sfx}")
    PVT = st["PVT"] = consts.tile([128, cw, INTER], F32, tag=f"PVT{sfx}", name=f"PVT{sfx}")
    # G2[s] = (0, G_s) interleaved pairs: the multiplier stream for every
    # FMA-scan (kill column 0 resets the running state per token column).
    G2 = st["G2"] = consts.tile([128, INTER, cw, 2], F32, tag=f"G2{sfx}", name=f"G2{sfx}")
    # G4[s] = (0, 1, G_s, 1) quads: R-scan stream fusing the split-R adds:
    # state over (r2p, t2, r1p, t1) ends at r1p+t1 + G*(r2p+t2).
    G4 = st["G4"] = consts.tile([128, INTER, cw, 4], F32, tag=f"G4{sfx}", name=f"G4{sfx}")
    # PRODB: col0 = 0 (scan kill), cols 1..16 = |O|+1 factors (cols beyond
    # the step's valid count stay at the 1.0 preset)
    PRODB = st["PRODB"] = consts.tile([128, cw, 17], F32, tag=f"PRODB{sfx}", name=f"PRODB{sfx}")
    # ZSEED: col0 = pv*PVT[s] (scan seed via op1=add), cols 1..16 = 0
    ZSEED = st["ZSEED"] = consts.tile([128, cw, 17], F32, tag=f"ZSEED{sfx}", name=f"ZSEED{sfx}")
    st["OUT"] = consts.tile([128, cw], F32, tag=f"OUT{sfx}", name=f"OUT{sfx}")
    st["VMAG"] = VMAG
    nc.vector.memset(PRODB, 1.0)
    nc.vector.memset(PRODB[:, :, 0], 0.0)
    nc.vector.memset(ZSEED, 0.0)
    nc.vector.memset(G2, 0.0)
    nc.vector.memset(G4, 0.0)
    g4ones = bass.AP(tensor=G4.tensor, offset=G4.offset + 1,
                     ap=[G4.ap[0], [4, INTER * cw], [2, 2]])
    nc.vector.memset(g4ones, 1.0)

    # sigmoid_and_others table set: Tanh + Sigmoid. Phase 1 only — both
    # chunks' tanh/sigmoid run back-to-back so the kernel needs one
    # sigmoid-set load and one ln/exp-set load total.
    nc.scalar.activation(VSIGN, PLAN[:, :, 8:24], ACTF.Tanh)
    nc.scalar.activation(G, PLAN[:, :, 116:124], ACTF.Sigmoid)
    return st


def _dag_init2(nc, ns, PLAN, st, sfx, cw):
    G, VSIGN, VMAG, LMD = st["G"], st["VSIGN"], st["VMAG"], st["LMD"]
    PVT = st["PVT"]
    DIFF = LMD[:, 0, :, :]
    LOGMAG = LMD[:, 1, :, :]
    # G2 pairs: (0, G_s) per step (col 0 left zero by the init memset)
    G2 = st["G2"]
    g2v = bass.AP(tensor=G2.tensor, offset=G2.offset + 1,
                  ap=[G2.ap[0], [2, cw], [2 * cw, INTER]])
    nc.vector.tensor_copy(out=g2v, in_=G[:, :, :])
    G4 = st["G4"]
    g4v = bass.AP(tensor=G4.tensor, offset=G4.offset + 2,
                  ap=[G4.ap[0], [4, cw], [4 * cw, INTER]])
    nc.vector.tensor_copy(out=g4v, in_=G[:, :, :])

    # Only slots 0-7 need init magnitudes; slots 8-15 are written by the steps
    # before first use, so their LMD entries start uninitialised.
    vabs = ns.tile([128, cw, INIT_SLOTS], F32, tag=f"vabs{sfx}")
    nc.vector.tensor_scalar(out=vabs.bitcast(U32), in0=PLAN[:, :, 0:8].bitcast(U32),
                            scalar1=0x7FFFFFFF, scalar2=None, op0=ALU.bitwise_and)
    nc.vector.tensor_scalar(out=VMAG, in0=vabs, scalar1=1e-12,
                            scalar2=None, op0=ALU.max)
    # natural_log_exp_and_others table set from here on
    nc.scalar.activation(LOGMAG[:, :, 0:8], VMAG, ACTF.Ln)
    sg0 = ns.tile([128, cw, INIT_SLOTS], F32, tag=f"sg0{sfx}")
    nc.vector.tensor_tensor(out=sg0, in0=VSIGN[:, :, 0:8], in1=VMAG,
                            op=ALU.mult)
    nc.vector.tensor_tensor(out=DIFF[:, :, 0:8], in0=sg0, in1=LOGMAG[:, :, 0:8],
                            op=ALU.subtract)

    # PV head init: prod of V_sign slots 0..7
    pva = ns.tile([128, cw, 4], F32, tag=f"pva{sfx}")
    nc.vector.tensor_tensor(out=pva, in0=VSIGN[:, :, 0:4], in1=VSIGN[:, :, 4:8], op=ALU.mult)
    pvb = ns.tile([128, cw, 2], F32, tag=f"pvb{sfx}")
    nc.vector.tensor_tensor(out=pvb, in0=pva[:, :, 0:2], in1=pva[:, :, 2:4], op=ALU.mult)
    pv = ns.tile([128, cw], F32, tag=f"pv{sfx}")
    nc.vector.tensor_tensor(out=pv, in0=pvb[:, :, 0], in1=pvb[:, :, 1], op=ALU.mult)
    st["pv"] = pv

    # PV tail suffix products: PVT[:, :, s] = prod_{j >= 8+s} V_sign_init[j]
    nc.vector.tensor_copy(out=PVT[:, :, INTER - 1], in_=VSIGN[:, :, NN - 1])
    for j in range(INTER - 2, -1, -1):
        nc.vector.tensor_tensor(out=PVT[:, :, j], in0=PVT[:, :, j + 1],
                                in1=VSIGN[:, :, 8 + j], op=ALU.mult)


def _dag_step(nc, ns, PLAN, st, sfx, cw, s):
    G, LMD, PVT = st["G"], st["LMD"], st["PVT"]
    G2 = st["G2"]
    DIFF = LMD[:, 0, :, :]
    LOGMAG = LMD[:, 1, :, :]
    v = INIT_SLOTS + s          # valid node count (mask: pos < v)
    O_s = PLAN[:, :, OPOFF[s]: OPOFF[s] + v]
    g2s = _flat(G2, 2 * cw, off=s * 2 * cw)   # (0, G_s) pairs

    def T(nm, shape=None, dt=F32):
        return ns.tile(shape or [128, cw], dt, tag=f"{nm}{sfx}", name=f"{nm}{sfx}")

    # --- R = r1 + G*r2 (quad FMA-scan fuses the split-R adds) -------------
    G4 = st["G4"]
    g4s = _flat(G4, 4 * cw, off=s * 4 * cw)   # (0, 1, G_s, 1) quads
    vp = v - 1 if (SPLIT_R and s > 0) else v
    QUAD = ns.tile([128, cw, 4], F32, tag=f"QUAD{sfx}", name=f"QUAD{sfx}")
    if s == 0:
        qz = bass.AP(tensor=QUAD.tensor, offset=QUAD.offset + 1,
                     ap=[QUAD.ap[0], [4, cw], [2, 2]])
        nc.vector.memset(qz, 0.0)
    m12 = T("m12", [128, cw, 2, NN])
    ob = bass.AP(tensor=O_s.tensor, offset=O_s.offset,
                 ap=[O_s.ap[0], O_s.ap[1], [0, 2], [O_s.ap[2][0], vp]])
    lv = bass.AP(tensor=LMD.tensor, offset=LMD.offset + 0,
                 ap=[LMD.ap[0], LMD.ap[2], LMD.ap[1], [LMD.ap[3][0], vp]])
    nc.vector.tensor_tensor(out=m12[:, :, :, :vp], in0=ob, in1=lv, op=ALU.mult)
    # (r2p, r1p) -> quad cols (0, 2)
    qr = bass.AP(tensor=QUAD.tensor, offset=QUAD.offset,
                 ap=[QUAD.ap[0], [4, cw], [2, 2]])
    nc.vector.tensor_reduce(out=qr, in_=m12[:, :, :, :vp], op=ALU.add, axis=AXX)
    if vp != v:
        # newest column's contribution -> quad cols (1, 3) (prefix computed
        # without it so the previous step's Ln/diff stays off launch deps)
        nnode = v - 1
        oc = bass.AP(tensor=O_s.tensor, offset=O_s.offset + nnode,
                     ap=[O_s.ap[0], O_s.ap[1], [0, 2]])
        lc = bass.AP(tensor=LMD.tensor, offset=LMD.offset + nnode,
                     ap=[LMD.ap[0], LMD.ap[2], LMD.ap[1]])
        qt = bass.AP(tensor=QUAD.tensor, offset=QUAD.offset + 1,
                     ap=[QUAD.ap[0], [4, cw], [2, 2]])
        nc.vector.tensor_tensor(out=qt, in0=oc, in1=lc, op=ALU.mult)
    rq = T("rq", [128, cw, 4])
    nc.vector.tensor_tensor_scan(out=_flat(rq, 4 * cw), data0=g4s,
                                 data1=_flat(QUAD, 4 * cw), initial=0.0,
                                 op0=ALU.mult, op1=ALU.add)
    R = rq[:, :, 3]

    # --- sign_prod = pv * PVT[s] * prod_{j<v}(|O_s|+1) via product scan ----
    # PRODB col0 = 0 kills the running state per token col; ZSEED col0 seeds
    # it with pv*PVT[s] via the op1=add leg.
    PRODB, ZSEED = st["PRODB"], st["ZSEED"]
    nc.vector._custom_dve(ABS1, out=PRODB[:, :, 1:1 + v], in0=O_s)
    nc.vector.tensor_tensor(out=ZSEED[:, :, 0], in0=st["pv"], in1=PVT[:, :, s],
                            op=ALU.mult)
    sscr = T("sscr", [128, cw, 17])
    nc.vector.tensor_tensor_scan(out=_flat(sscr, 17 * cw),
                                 data0=_flat(PRODB, 17 * cw),
                                 data1=_flat(ZSEED, 17 * cw), initial=0.0,
                                 op0=ALU.mult, op1=ALU.add)
    SP = sscr[:, :, 16]

    # --- exp batch: (exp(-2e4|R|), exp(-2e4|SP|)) pairs + exp(min(R,23)) ---
    axp = T("axp", [128, cw, 2])
    nc.vector._custom_dve(ABSM, out=axp[:, :, 0], in0=R, imm2=-2.0e4)
    nc.vector._custom_dve(ABSM, out=axp[:, :, 1], in0=SP, imm2=-2.0e4)
    xr = T("xr")
    nc.vector.tensor_scalar(out=xr, in0=R, scalar1=LOG_CLAMP, scalar2=None,
                            op0=ALU.min)
    eo = T("eo", [128, cw, 2])
    nc.scalar.activation(eo, axp, ACTF.Exp)
    pairT = T("pairT", [128, cw, 2])
    er = pairT[:, :, 1]
    nc.scalar.activation(er, xr, ACTF.Exp)

    # --- tanh(x/1e-4) pair: (1-e)/(1+e) via 2-Newton custom ops + sign ----
    y1 = T("y1", [128, cw, 2])
    nc.vector._custom_dve(TANH_A, out=y1, in0=eo, s0=_TANH_C0, s1=_TANH_C1,
                          imm2=2.0)
    uu = T("uu", [128, cw, 2])
    nc.vector._custom_dve(TANH_B, out=uu, in0=eo, in1=y1, s0=2.0)
    sgp = T("sgp", [128, cw, 2], U32)
    nc.vector.tensor_scalar(out=sgp[:, :, 0], in0=R.bitcast(U32),
                            scalar1=0x80000000, scalar2=None, op0=ALU.bitwise_and)
    nc.vector.tensor_scalar(out=sgp[:, :, 1], in0=SP.bitcast(U32),
                            scalar1=0x80000000, scalar2=None, op0=ALU.bitwise_and)
    vspair = T("vspair", [128, cw, 2])
    nc.vector.tensor_tensor(out=vspair.bitcast(U32), in0=uu.bitcast(U32),
                            in1=sgp, op=ALU.bitwise_xor)

    # --- Vs = lgs + G*(lin - lgs) via in-place diff + FMA-scan ------------
    nc.vector.tensor_tensor(out=vspair[:, :, 0], in0=vspair[:, :, 0],
                            in1=vspair[:, :, 1], op=ALU.subtract)
    vsscr = T("vsscr", [128, cw, 2])
    nc.vector.tensor_tensor_scan(out=_flat(vsscr, 2 * cw), data0=g2s,
                                 data1=_flat(vspair, 2 * cw), initial=0.0,
                                 op0=ALU.mult, op1=ALU.add)
    vs = vsscr[:, :, 1]

    # --- Vm = er + G*(|R| - er) -------------------------------------------
    # d = |R| - er recovered from the prescaled exp input: (a - er*c0)*c2
    # with a = -2e4|R|, c0 = -2e4, c2 = -1/2e4.
    nc.vector._custom_dve(LN_BWD_DX_ANT, out=pairT[:, :, 0], in0=axp[:, :, 0],
                          in1=er, s0=-2.0e4, s1=0.0, imm2=-5.0e-5)
    vmscr = T("vmscr", [128, cw, 2])
    nc.vector.tensor_tensor_scan(out=_flat(vmscr, 2 * cw), data0=g2s,
                                 data1=_flat(pairT, 2 * cw), initial=0.0,
                                 op0=ALU.mult, op1=ALU.add)
    vm = vmscr[:, :, 1]

    if s == INTER - 1:
        nc.vector.tensor_tensor(out=st["OUT"], in0=vs, in1=vm, op=ALU.mult)
    else:
        idx = INIT_SLOTS + s
        sgnew = T("sgnew")
        nc.vector.tensor_tensor(out=sgnew, in0=vs, in1=vm, op=ALU.mult)
        vmc = T("vmc")
        nc.vector.tensor_scalar(out=vmc, in0=vm, scalar1=1e-12, scalar2=None,
                                op0=ALU.max)
        nc.scalar.activation(LOGMAG[:, :, idx], vmc, ACTF.Ln)
        nc.vector.tensor_tensor(out=DIFF[:, :, idx], in0=sgnew,
                                in1=LOGMAG[:, :, idx], op=ALU.subtract)
        pv_next = ns.tile([128, cw], F32, tag=f"pv{sfx}")
        nc.vector.tensor_tensor(out=pv_next, in0=st["pv"], in1=vs, op=ALU.mult)
        st["pv"] = pv_next


def _get_nc():
    if "nc" not in _CACHE:
        _CACHE["nc"] = _build()
    return _CACHE["nc"]


def _prep_inputs(hidden, W_init, b_init, W_op, b_op, W_gate, b_gate):
    hidden = np.ascontiguousarray(np.asarray(hidden, np.float32)).reshape(NTOK, H)
    W_init = np.asarray(W_init, np.float32)
    W_op = np.asarray(W_op, np.float32)
    W_gate = np.asarray(W_gate, np.float32)
    b_init = np.asarray(b_init, np.float32)
    b_op = np.asarray(b_op, np.float32)
    b_gate = np.asarray(b_gate, np.float32)

    # Pruned output packing: 8 init mags, 16 init signs, 92 masked-op rows,
    # 8 gates (see NF/OPOFF above).
    rows_w = [W_init[0:8], W_init[16:32]]
    rows_b = [b_init[0:8], b_init[16:32]]
    for s in range(INTER):
        rows_w.append(W_op[s * NN: s * NN + INIT_SLOTS + s])
        rows_b.append(b_op[s * NN: s * NN + INIT_SLOTS + s])
    rows_w.append(W_gate)
    rows_b.append(b_gate)
    Wcat = np.concatenate(rows_w, axis=0)                              # [124, H]
    bcat = np.concatenate(rows_b)                                      # [124]
    assert Wcat.shape[0] == NF

    WT = np.ascontiguousarray(Wcat.T)                                  # [H, 124]
    Wh = WT.astype(np.float16)
    Wl = ((WT - Wh.astype(np.float32)) * SCL).astype(np.float16)
    wt = np.concatenate([Wh, Wl], axis=1)                              # [H, 248]

    bh = bcat.astype(np.float16)
    bl = ((bcat - bh.astype(np.float32)) * SCL).astype(np.float16)
    bias = np.concatenate([bh, bl])[None, :]                           # [1, 248]

    in_maps = []
    for c in range(NCORES):
        shard = hidden[c * TPC:(c + 1) * TPC]                          # [2048, H]
        hT = np.ascontiguousarray(shard.T)                             # [H, 2048]
        fh = hT.astype(np.float16)
        fl = ((hT - fh.astype(np.float32)) * SCL).astype(np.float16)
        # [NTILE, 128p, KCH, 256]: partition-major so each partition's DMA
        # line is KCH*256*2B = 8KB contiguous (big packets, ~22GB/s/engine).
        # Per tile i, [p, k, 0:128] = hi of tokens, [p, k, 128:256] = lo,
        # where h = k*128 + p.
        comb = np.empty((NTILE, 128, KCH, 256), np.float16)
        fh4 = fh.reshape(KCH, 128, NTILE, 128)
        fl4 = fl.reshape(KCH, 128, NTILE, 128)
        comb[:, :, :, 0:128] = fh4.transpose(2, 1, 0, 3)
        comb[:, :, :, 128:256] = fl4.transpose(2, 1, 0, 3)
        # group 2 tiles per DMA: [grp, 128p, t, KCH, 256] -> 16KB/partition line
        comb = np.ascontiguousarray(
            comb.reshape(NTILE // 2, 2, 128, KCH, 256).transpose(0, 2, 1, 3, 4))
        in_maps.append({"hf": comb, "wt": wt, "bias": bias})
    return in_maps


def _run(in_maps, **kwargs):
    nc = _get_nc()
    return run_bass_kernel_spmd(nc, in_maps, core_ids=list(range(NCORES)), **kwargs)


def _assemble(results):
    out = np.empty((NTOK,), np.float32)
    for c in range(NCORES):
        out[c * TPC:(c + 1) * TPC] = results[c]["out"].T.reshape(TPC)
    return out.reshape(B, T)


def kernel(**inputs):
    in_maps = _prep_inputs(**inputs)
    res = _run(in_maps)
    return _assemble(res.results)


def kernel_traced(**inputs):
    """Like kernel() but with NTFF tracing; returns (output, BassKernelResults)."""
    in_maps = _prep_inputs(**inputs)
    res = _run(in_maps, trace=True)
    return _assemble(res.results), res



# revision 34
# speedup vs baseline: 1.1623x; 1.1623x over previous
"""Trainium2 Bass kernel for nn_NewDAGExecutor (plan-predictor matmul + 8-step DAG).

Strategy (8 NeuronCores, data-parallel over the 16384 tokens, 2048 tokens/core):
  - Host: transpose each core's token shard to [H, tok], split fp32 into an
    exact fp16 hi/lo pair (lo scaled by 2^11), same for the concatenated
    weight matrix W = [W_init; W_op; W_gate] (168 outputs) and biases.
  - Device: plan = hidden @ W.T + b computed as 3 fp16 matmul passes
    (hi*Whi -> PSUM1, hi*Wlo + lo*Whi -> PSUM2, combine P1 + 2^-11*P2),
    which is fp32-accurate (fp16 products are exact in the fp32 MACs; the
    dropped lo*lo term is ~2^-24 relative). Tokens ride the PSUM partition
    axis so the DAG math lands in token-major layout with no transpose.
  - The 8 sequential DAG steps run wide across all 2048 tokens/core
    ([128 partitions x 16 tile-columns]) on DVE/ACT. tanh(x*1e4) is computed
    via exp to keep every per-step ACT call inside the natural_log_exp table
    set (no per-step table switches); the sign is restored with bitwise ops.
"""

import numpy as np

import concourse.bacc as bacc
import concourse.bass as bass
import concourse.tile as tile
import concourse.mybir as mybir
from concourse.bass_utils import run_bass_kernel_spmd

import concourse.dve_ops as _dve_ops_mod
from concourse.dve_ops import DveOp as _DveOp, LN_BWD_DX_ANT
from concourse.dve_spec import (
    Spec as _Spec, Src0 as _S0, Src1 as _S1, C0 as _C0, C1 as _C1, C2 as _C2,
    Zero as _Z, One as _One, lower as _dve_lower, _has_src1, maxx as _maxx,
)
from concourse.dve_uop import DveOpSpec as _DveOpSpec


def _register_dve_op(name, spec, subdim=False):
    """Register a custom DVE op at import time (documented extension point:
    dve_ops.OPS + the name->row map; sha computed here so compile()'s
    drift check is self-consistent)."""
    if name in _dve_ops_mod._SUB_OPCODE_FOR_NAME:
        for op in _dve_ops_mod.OPS:
            if op.name == name:
                return op
    opcode = _dve_ops_mod._CUSTOM_DVE_ROW_BASE + len(_dve_ops_mod.OPS)
    shas = {}
    for ver in ("v3", "v4"):
        s = _DveOpSpec(name=name, opcode=opcode, uops=_dve_lower(spec, ver=ver),
                       rd1_en=_has_src1(spec))
        shas[ver] = s.sha(ver)
    op = _DveOp(name, spec, subdim, shas)
    _dve_ops_mod.OPS.append(op)
    _dve_ops_mod._SUB_OPCODE_FOR_NAME[name] = opcode
    _dve_ops_mod.CUSTOM_DVE_SPECS[name] = spec
    return op


# |x| + 1 (for the sign_prod factors)
ABS1 = _register_dve_op("ANT_ABS1", _Spec(
    body=_maxx(_S0, _Z - _S0) + _One,
    reference=lambda in0, in1, s0, s1, imm2: np.abs(in0.astype(np.float32)) + 1.0,
))

# imm2 * |x| (prescale for exp(-2e4|x|))
ABSM = _register_dve_op("ANT_ABSM", _Spec(
    body=_maxx(_S0, _Z - _S0) * _C2,
    reference=lambda in0, in1, s0, s1, imm2: np.abs(in0.astype(np.float32)) * imm2,
))

# Newton reciprocal of d = 1+e, e in [0,1]: minimax linear seed on [1,2]
# (a=24/17, b=-8/17, seed rel err 1/17) + one Newton pass -> y1.
_d_node = _One + _S0
_y0_node = _d_node * _C1 + _C0
TANH_A = _register_dve_op("ANT_TANH_A", _Spec(
    body=_y0_node * (_C2 - _d_node * _y0_node),
    reference=lambda in0, in1, s0, s1, imm2: (
        lambda d: (lambda y0: y0 * (imm2 - d * y0))(d * s1 + s0))(
            1.0 + in0.astype(np.float32)),
))

# Second Newton pass + (1-e) numerator: out = (1-e) * y1*(2 - (1+e)*y1)
TANH_B = _register_dve_op("ANT_TANH_B", _Spec(
    body=(_One - _S0) * (_S1 * (_C0 - _d_node * _S1)),
    reference=lambda in0, in1, s0, s1, imm2: (
        (1.0 - in0.astype(np.float32)) *
        (in1 * (s0 - (1.0 + in0.astype(np.float32)) * in1))),
))

_TANH_C0 = 24.0 / 17.0
_TANH_C1 = -8.0 / 17.0

# Pin ACT table-set choice: the greedy first-match in insert_act_table_loads
# would alternate natural_log <-> exp_and_others every DAG step (2 x 2.7us per
# step). Advertise ln/exp only in natural_log_exp_and_others and tanh/sigmoid
# only in sigmoid_and_others so the whole kernel needs exactly 2 table loads.
_ORIG_GAT = bacc.get_activation_tables


def _pinned_activation_tables(arch):
    tables = _ORIG_GAT(arch)
    LN = mybir.ActivationFunctionType.Ln
    EXP = mybir.ActivationFunctionType.Exp
    TANH = mybir.ActivationFunctionType.Tanh
    SIG = mybir.ActivationFunctionType.Sigmoid
    for name, funcs in tables.items():
        if name != "natural_log_exp_and_others":
            funcs.discard(LN)
            funcs.discard(EXP)
        if name != "sigmoid_and_others":
            funcs.discard(TANH)
            funcs.discard(SIG)
    return tables


bacc.get_activation_tables = _pinned_activation_tables

F32 = mybir.dt.float32
F16 = mybir.dt.float16
U32 = mybir.dt.uint32
ALU = mybir.AluOpType
ACTF = mybir.ActivationFunctionType
AXX = mybir.AxisListType.X

NCORES = 8
B, T, H = 4, 4096, 2048
NTOK = B * T                    # 16384
TPC = NTOK // NCORES            # 2048 tokens per core
NTILE = TPC // 128              # 16 token tiles per core
KCH = H // 128                  # 16 contraction chunks
NN = 16                         # DAG nodes
INTER = 8                       # steps
INIT_SLOTS = 8
# Pruned plan layout: only outputs the DAG actually reads.
#   [0:8]    init magnitudes for slots 0-7 (slots 8-15 are dead: they are
#            rewritten at step s=j-8 before any step can read them)
#   [8:24]   init signs, all 16 slots (sign_prod reads every slot)
#   [24:116] op rows packed: step s contributes its first 8+s nodes only
#            (the causal mask zeroes the rest)
#   [116:124] gates
OPOFF = [24]
for _s in range(1, INTER):
    OPOFF.append(OPOFF[-1] + INIT_SLOTS + _s - 1)
NF = 124
LOG_CLAMP = 23.026
SCL = 2048.0                    # 2^11 lo-part scale
ISCL = 1.0 / SCL
NCHUNKS = 2                     # DAG token-chunks (overlap DAG with later matmuls)
BATCH_EXP = True                # batch et+er exps into one ACT call per step
SPLIT_R = True                  # prefix/last-col split of the R reductions

_CACHE = {}


def _build(repeats=1, parts="all"):
    nc = bacc.Bacc("TRN2", target_bir_lowering=False, debug=False)

    hf_d = nc.dram_tensor("hf", [NTILE // 2, 128, 2, KCH, 256], F16,
                          kind="ExternalInput")
    wt_d = nc.dram_tensor("wt", [H, 2 * NF], F16, kind="ExternalInput")
    bias_d = nc.dram_tensor("bias", [1, 2 * NF], F16, kind="ExternalInput")
    out_d = nc.dram_tensor("out", [128, NTILE], F32, kind="ExternalOutput")

    with tile.TileContext(nc) as tc:
        with tc.tile_pool(name="consts", bufs=1) as consts, \
             tc.tile_pool(name="hfp", bufs=3) as hfp, \
             tc.tile_pool(name="evp", bufs=3) as evp, \
             tc.tile_pool(name="ns", bufs=2) as ns, \
             tc.tile_pool(name="pp", bufs=3, space="PSUM") as pp:

            wt_sb = consts.tile([128, KCH, 2 * NF], F16)
            nc.sync.dma_start(out=wt_sb, in_=wt_d.rearrange("(k p) f -> p k f", p=128))
            bias_sb = consts.tile([1, 2 * NF], F16)
            nc.sync.dma_start(out=bias_sb, in_=bias_d[:, :])
            ones = consts.tile([1, 128], F16)
            nc.vector.memset(ones, 1.0)

            for _rep in range(repeats):
                _emit_body(nc, tc, consts, hfp, evp, ns, pp,
                           hf_d, wt_sb, bias_sb, ones, out_d, parts=parts)

    nc.compile()
    return nc


def _emit_body(nc, tc, consts, hfp, evp, ns, pp, hf_d, wt_sb, bias_sb, ones, out_d,
               parts="all"):
    cw = NTILE // NCHUNKS
    chunks = [(c * cw, (c + 1) * cw, chr(65 + c)) for c in range(NCHUNKS)]
    plan = {sfx: consts.tile([128, cw, NF], F32, tag=f"plan{sfx}", name=f"plan{sfx}")
            for _, _, sfx in chunks}

    # ---- plan predictor: 3-pass fp16 matmul per token tile ----
    if parts == "dag":
        for _, _, sfx in chunks:
            nc.vector.memset(plan[sfx], 0.5)
    for grp in range(NTILE // 2) if parts != "dag" else []:
        hf_sb = hfp.tile([128, 2, KCH, 256], F16, tag="hf")
        nc.sync.dma_start(out=hf_sb, in_=hf_d[grp])
        for t in range(2):
            i = grp * 2 + t
            lo_t, _, sfx = chunks[i // cw]
            p12 = pp.tile([128, 2 * NF], F32, tag="p12")
            nc.tensor.matmul(p12, ones[:, :], bias_sb[:, :], start=True, stop=False)
            for k in range(KCH):
                hi = hf_sb[:, t, k, 0:128]
                lo = hf_sb[:, t, k, 128:256]
                wh = wt_sb[:, k, 0:NF]
                whl = wt_sb[:, k, :]
                # hi * [Wh | Wl] in one N=248 matmul; lo * Wh into the hi-Wl half
                nc.tensor.matmul(p12, hi, whl, start=False, stop=False)
                nc.tensor.matmul(p12[:, NF:2 * NF], lo, wh, start=False,
                                 stop=(k == KCH - 1), skip_group_check=True)
            tmp = evp.tile([128, NF], F32, tag="ev")
            nc.scalar.activation(tmp, p12[:, NF:2 * NF], ACTF.Copy, bias=0.0,
                                 scale=ISCL)
            nc.vector.tensor_tensor(out=plan[sfx][:, i - lo_t, :],
                                    in0=p12[:, 0:NF], in1=tmp, op=ALU.add)

    if parts == "mm":
        nc.sync.dma_start(out=out_d[:, :], in_=plan[chunks[0][2]][:, 0, 0:NTILE])
        return

    st = {}
    for _, _, sfx in chunks:
        st[sfx] = _dag_init(nc, consts, ns, plan[sfx], sfx, cw)
    for _, _, sfx in chunks:
        _dag_init2(nc, ns, plan[sfx], st[sfx], sfx, cw)
    for s in range(INTER):
        for _, _, sfx in chunks:
            _dag_step(nc, ns, plan[sfx], st[sfx], sfx, cw, s)
    for lo_t, hi_t, sfx in chunks:
        nc.sync.dma_start(out=out_d[:, lo_t:hi_t], in_=st[sfx]["OUT"])


def _flat(t, n, off=0):
    """[P, ...] contiguous tile -> [P, n] flat free view at element offset."""
    return bass.AP(tensor=t.tensor, offset=t.offset + off, ap=[t.ap[0], [1, n]])


def _dag_init(nc, consts, ns, PLAN, sfx, cw):
    st = {}
    G = st["G"] = consts.tile([128, cw, INTER], F32, tag=f"G{sfx}", name=f"G{sfx}")
    VSIGN = st["VSIGN"] = consts.tile([128, cw, NN], F32, tag=f"VSIGN{sfx}", name=f"VSIGN{sfx}")
    VMAG = consts.tile([128, cw, INIT_SLOTS], F32, tag=f"VMAG{sfx}")
    # LMD rows: 0 = DIFF (signed - log), 1 = LOGMAG — diff first so the
    # m12 reduce emits (r2, r1) pairs ready for the R FMA-scan.
    LMD = st["LMD"] = consts.tile([128, 2, cw, NN], F32, tag=f"LMD{sfx}", name=f"LMD{sfx}")
    PVT = st["PVT"] = consts.tile([128, cw, INTER], F32, tag=f"PVT{sfx}", name=f"PVT{sfx}")
    # G2[s] = (0, G_s) interleaved pairs: the multiplier stream for every
    # FMA-scan (kill column 0 resets the running state per token column).
    G2 = st["G2"] = consts.tile([128, INTER, cw, 2], F32, tag=f"G2{sfx}", name=f"G2{sfx}")
    # G4[s] = (0, 1, G_s, 1) quads: R-scan stream fusing the split-R adds:
    # state over (r2p, t2, r1p, t1) ends at r1p+t1 + G*(r2p+t2).
    G4 = st["G4"] = consts.tile([128, INTER, cw, 4], F32, tag=f"G4{sfx}", name=f"G4{sfx}")
    # PRODB: col0 = 0 (scan kill), cols 1..16 = |O|+1 factors (cols beyond
    # the step's valid count stay at the 1.0 preset)
    PRODB = st["PRODB"] = consts.tile([128, cw, 17], F32, tag=f"PRODB{sfx}", name=f"PRODB{sfx}")
    # ZSEED: col0 = pv*PVT[s] (scan seed via op1=add), cols 1..16 = 0
    ZSEED = st["ZSEED"] = consts.tile([128, cw, 17], F32, tag=f"ZSEED{sfx}", name=f"ZSEED{sfx}")
    st["OUT"] = consts.tile([128, cw], F32, tag=f"OUT{sfx}", name=f"OUT{sfx}")
    st["VMAG"] = VMAG
    nc.vector.memset(PRODB, 1.0)
    nc.vector.memset(PRODB[:, :, 0], 0.0)
    nc.vector.memset(ZSEED, 0.0)
    nc.vector.memset(G2, 0.0)
    nc.vector.memset(G4, 0.0)
    g4ones = bass.AP(tensor=G4.tensor, offset=G4.offset + 1,
                     ap=[G4.ap[0], [4, INTER * cw], [2, 2]])
    nc.vector.memset(g4ones, 1.0)

    # sigmoid_and_others table set: Tanh + Sigmoid. Phase 1 only — both
    # chunks' tanh/sigmoid run back-to-back so the kernel needs one
    # sigmoid-set load and one ln/exp-set load total.
    nc.scalar.activation(VSIGN, PLAN[:, :, 8:24], ACTF.Tanh)
    nc.scalar.activation(G, PLAN[:, :, 116:124], ACTF.Sigmoid)
    return st


def _dag_init2(nc, ns, PLAN, st, sfx, cw):
    G, VSIGN, VMAG, LMD = st["G"], st["VSIGN"], st["VMAG"], st["LMD"]
    PVT = st["PVT"]
    DIFF = LMD[:, 0, :, :]
    LOGMAG = LMD[:, 1, :, :]
    # G2 pairs: (0, G_s) per step (col 0 left zero by the init memset)
    G2 = st["G2"]
    g2v = bass.AP(tensor=G2.tensor, offset=G2.offset + 1,
                  ap=[G2.ap[0], [2, cw], [2 * cw, INTER]])
    nc.vector.tensor_copy(out=g2v, in_=G[:, :, :])
    G4 = st["G4"]
    g4v = bass.AP(tensor=G4.tensor, offset=G4.offset + 2,
                  ap=[G4.ap[0], [4, cw], [4 * cw, INTER]])
    nc.vector.tensor_copy(out=g4v, in_=G[:, :, :])

    # Only slots 0-7 need init magnitudes; slots 8-15 are written by the steps
    # before first use, so their LMD entries start uninitialised.
    vabs = ns.tile([128, cw, INIT_SLOTS], F32, tag=f"vabs{sfx}")
    nc.vector.tensor_scalar(out=vabs.bitcast(U32), in0=PLAN[:, :, 0:8].bitcast(U32),
                            scalar1=0x7FFFFFFF, scalar2=None, op0=ALU.bitwise_and)
    nc.vector.tensor_scalar(out=VMAG, in0=vabs, scalar1=1e-12,
                            scalar2=None, op0=ALU.max)
    # natural_log_exp_and_others table set from here on
    nc.scalar.activation(LOGMAG[:, :, 0:8], VMAG, ACTF.Ln)
    sg0 = ns.tile([128, cw, INIT_SLOTS], F32, tag=f"sg0{sfx}")
    nc.vector.tensor_tensor(out=sg0, in0=VSIGN[:, :, 0:8], in1=VMAG,
                            op=ALU.mult)
    nc.vector.tensor_tensor(out=DIFF[:, :, 0:8], in0=sg0, in1=LOGMAG[:, :, 0:8],
                            op=ALU.subtract)

    # PV head init: prod of V_sign slots 0..7
    pva = ns.tile([128, cw, 4], F32, tag=f"pva{sfx}")
    nc.vector.tensor_tensor(out=pva, in0=VSIGN[:, :, 0:4], in1=VSIGN[:, :, 4:8], op=ALU.mult)
    pvb = ns.tile([128, cw, 2], F32, tag=f"pvb{sfx}")
    nc.vector.tensor_tensor(out=pvb, in0=pva[:, :, 0:2], in1=pva[:, :, 2:4], op=ALU.mult)
    pv = ns.tile([128, cw], F32, tag=f"pv{sfx}")
    nc.vector.tensor_tensor(out=pv, in0=pvb[:, :, 0], in1=pvb[:, :, 1], op=ALU.mult)
    st["pv"] = pv

    # PV tail suffix products: PVT[:, :, s] = prod_{j >= 8+s} V_sign_init[j]
    nc.vector.tensor_copy(out=PVT[:, :, INTER - 1], in_=VSIGN[:, :, NN - 1])
    for j in range(INTER - 2, -1, -1):
        nc.vector.tensor_tensor(out=PVT[:, :, j], in0=PVT[:, :, j + 1],
                                in1=VSIGN[:, :, 8 + j], op=ALU.mult)


def _dag_step(nc, ns, PLAN, st, sfx, cw, s):
    G, LMD, PVT = st["G"], st["LMD"], st["PVT"]
    G2 = st["G2"]
    DIFF = LMD[:, 0, :, :]
    LOGMAG = LMD[:, 1, :, :]
    v = INIT_SLOTS + s          # valid node count (mask: pos < v)
    O_s = PLAN[:, :, OPOFF[s]: OPOFF[s] + v]
    g2s = _flat(G2, 2 * cw, off=s * 2 * cw)   # (0, G_s) pairs

    def T(nm, shape=None, dt=F32):
        return ns.tile(shape or [128, cw], dt, tag=f"{nm}{sfx}", name=f"{nm}{sfx}")

    # --- R = r1 + G*r2 ----------------------------------------------------
    vp = v - 1 if (SPLIT_R and s > 0) else v
    m12 = T("m12", [128, cw, 2, NN])
    ob = bass.AP(tensor=O_s.tensor, offset=O_s.offset,
                 ap=[O_s.ap[0], O_s.ap[1], [0, 2], [O_s.ap[2][0], vp]])
    lv = bass.AP(tensor=LMD.tensor, offset=LMD.offset + 0,
                 ap=[LMD.ap[0], LMD.ap[2], LMD.ap[1], [LMD.ap[3][0], vp]])
    nc.vector.tensor_tensor(out=m12[:, :, :, :vp], in0=ob, in1=lv, op=ALU.mult)
    r12 = T("r12", [128, cw, 2])
    nc.vector.tensor_reduce(out=r12, in_=m12[:, :, :, :vp], op=ALU.add, axis=AXX)
    if vp != v:
        nnode = v - 1
        oc = bass.AP(tensor=O_s.tensor, offset=O_s.offset + nnode,
                     ap=[O_s.ap[0], O_s.ap[1], [0, 2]])
        lc = bass.AP(tensor=LMD.tensor, offset=LMD.offset + nnode,
                     ap=[LMD.ap[0], LMD.ap[2], LMD.ap[1]])
        tnewc = T("tnewc", [128, cw, 2])
        nc.vector.tensor_tensor(out=tnewc, in0=oc, in1=lc, op=ALU.mult)
        r12f = T("r12f", [128, cw, 2])
        nc.vector.tensor_tensor(out=r12f, in0=r12, in1=tnewc, op=ALU.add)
        r12 = r12f
    rscr = T("rscr", [128, cw, 2])
    nc.vector.tensor_tensor_scan(out=_flat(rscr, 2 * cw), data0=g2s,
                                 data1=_flat(r12, 2 * cw), initial=0.0,
                                 op0=ALU.mult, op1=ALU.add)
    R = rscr[:, :, 1]

    # --- sign_prod = pv * PVT[s] * prod_{j<v}(|O_s|+1) via product scan ----
    # PRODB col0 = 0 kills the running state per token col; ZSEED col0 seeds
    # it with pv*PVT[s] via the op1=add leg.
    PRODB, ZSEED = st["PRODB"], st["ZSEED"]
    nc.vector._custom_dve(ABS1, out=PRODB[:, :, 1:1 + v], in0=O_s)
    nc.vector.tensor_tensor(out=ZSEED[:, :, 0], in0=st["pv"], in1=PVT[:, :, s],
                            op=ALU.mult)
    sscr = T("sscr", [128, cw, 17])
    nc.vector.tensor_tensor_scan(out=_flat(sscr, 17 * cw),
                                 data0=_flat(PRODB, 17 * cw),
                                 data1=_flat(ZSEED, 17 * cw), initial=0.0,
                                 op0=ALU.mult, op1=ALU.add)
    SP = sscr[:, :, 16]

    # --- exp batch: (exp(-2e4|R|), exp(-2e4|SP|)) pairs + exp(min(R,23)) ---
    axp = T("axp", [128, cw, 2])
    nc.vector._custom_dve(ABSM, out=axp[:, :, 0], in0=R, imm2=-2.0e4)
    nc.vector._custom_dve(ABSM, out=axp[:, :, 1], in0=SP, imm2=-2.0e4)
    xr = T("xr")
    nc.vector.tensor_scalar(out=xr, in0=R, scalar1=LOG_CLAMP, scalar2=None,
                            op0=ALU.min)
    eo = T("eo", [128, cw, 2])
    nc.scalar.activation(eo, axp, ACTF.Exp)
    pairT = T("pairT", [128, cw, 2])
    er = pairT[:, :, 1]
    nc.scalar.activation(er, xr, ACTF.Exp)

    # --- tanh(x/1e-4) pair: (1-e)/(1+e) via 2-Newton custom ops + sign ----
    y1 = T("y1", [128, cw, 2])
    nc.vector._custom_dve(TANH_A, out=y1, in0=eo, s0=_TANH_C0, s1=_TANH_C1,
                          imm2=2.0)
    uu = T("uu", [128, cw, 2])
    nc.vector._custom_dve(TANH_B, out=uu, in0=eo, in1=y1, s0=2.0)
    sgp = T("sgp", [128, cw, 2], U32)
    nc.vector.tensor_scalar(out=sgp[:, :, 0], in0=R.bitcast(U32),
                            scalar1=0x80000000, scalar2=None, op0=ALU.bitwise_and)
    nc.vector.tensor_scalar(out=sgp[:, :, 1], in0=SP.bitcast(U32),
                            scalar1=0x80000000, scalar2=None, op0=ALU.bitwise_and)
    vspair = T("vspair", [128, cw, 2])
    nc.vector.tensor_tensor(out=vspair.bitcast(U32), in0=uu.bitcast(U32),
                            in1=sgp, op=ALU.bitwise_xor)

    # --- Vs = lgs + G*(lin - lgs) via in-place diff + FMA-scan ------------
    nc.vector.tensor_tensor(out=vspair[:, :, 0], in0=vspair[:, :, 0],
                            in1=vspair[:, :, 1], op=ALU.subtract)
    vsscr = T("vsscr", [128, cw, 2])
    nc.vector.tensor_tensor_scan(out=_flat(vsscr, 2 * cw), data0=g2s,
                                 data1=_flat(vspair, 2 * cw), initial=0.0,
                                 op0=ALU.mult, op1=ALU.add)
    vs = vsscr[:, :, 1]

    # --- Vm = er + G*(|R| - er) -------------------------------------------
    # d = |R| - er recovered from the prescaled exp input: (a - er*c0)*c2
    # with a = -2e4|R|, c0 = -2e4, c2 = -1/2e4.
    nc.vector._custom_dve(LN_BWD_DX_ANT, out=pairT[:, :, 0], in0=axp[:, :, 0],
                          in1=er, s0=-2.0e4, s1=0.0, imm2=-5.0e-5)
    vmscr = T("vmscr", [128, cw, 2])
    nc.vector.tensor_tensor_scan(out=_flat(vmscr, 2 * cw), data0=g2s,
                                 data1=_flat(pairT, 2 * cw), initial=0.0,
                                 op0=ALU.mult, op1=ALU.add)
    vm = vmscr[:, :, 1]

    if s == INTER - 1:
        nc.vector.tensor_tensor(out=st["OUT"], in0=vs, in1=vm, op=ALU.mult)
    else:
        idx = INIT_SLOTS + s
        sgnew = T("sgnew")
        nc.vector.tensor_tensor(out=sgnew, in0=vs, in1=vm, op=ALU.mult)
        vmc = T("vmc")
        nc.vector.tensor_scalar(out=vmc, in0=vm, scalar1=1e-12, scalar2=None,
                                op0=ALU.max)
        nc.scalar.activation(LOGMAG[:, :, idx], vmc, ACTF.Ln)
        nc.vector.tensor_tensor(out=DIFF[:, :, idx], in0=sgnew,
                                in1=LOGMAG[:, :, idx], op=ALU.subtract)
        pv_next = ns.tile([128, cw], F32, tag=f"pv{sfx}")
        nc.vector.tensor_tensor(out=pv_next, in0=st["pv"], in1=vs, op=ALU.mult)
        st["pv"] = pv_next


def _get_nc():
    if "nc" not in _CACHE:
        _CACHE["nc"] = _build()
    return _CACHE["nc"]


def _prep_inputs(hidden, W_init, b_init, W_op, b_op, W_gate, b_gate):
    hidden = np.ascontiguousarray(np.asarray(hidden, np.float32)).reshape(NTOK, H)
    W_init = np.asarray(W_init, np.float32)
    W_op = np.asarray(W_op, np.float32)
    W_gate = np.asarray(W_gate, np.float32)
    b_init = np.asarray(b_init, np.float32)
    b_op = np.asarray(b_op, np.float32)
    b_gate = np.asarray(b_gate, np.float32)

    # Pruned output packing: 8 init mags, 16 init signs, 92 masked-op rows,
    # 8 gates (see NF/OPOFF above).
    rows_w = [W_init[0:8], W_init[16:32]]
    rows_b = [b_init[0:8], b_init[16:32]]
    for s in range(INTER):
        rows_w.append(W_op[s * NN: s * NN + INIT_SLOTS + s])
        rows_b.append(b_op[s * NN: s * NN + INIT_SLOTS + s])
    rows_w.append(W_gate)
    rows_b.append(b_gate)
    Wcat = np.concatenate(rows_w, axis=0)                              # [124, H]
    bcat = np.concatenate(rows_b)                                      # [124]
    assert Wcat.shape[0] == NF

    WT = np.ascontiguousarray(Wcat.T)                                  # [H, 124]
    Wh = WT.astype(np.float16)
    Wl = ((WT - Wh.astype(np.float32)) * SCL).astype(np.float16)
    wt = np.concatenate([Wh, Wl], axis=1)                              # [H, 248]

    bh = bcat.astype(np.float16)
    bl = ((bcat - bh.astype(np.float32)) * SCL).astype(np.float16)
    bias = np.concatenate([bh, bl])[None, :]                           # [1, 248]

    in_maps = []
    for c in range(NCORES):
        shard = hidden[c * TPC:(c + 1) * TPC]                          # [2048, H]
        hT = np.ascontiguousarray(shard.T)                             # [H, 2048]
        fh = hT.astype(np.float16)
        fl = ((hT - fh.astype(np.float32)) * SCL).astype(np.float16)
        # [NTILE, 128p, KCH, 256]: partition-major so each partition's DMA
        # line is KCH*256*2B = 8KB contiguous (big packets, ~22GB/s/engine).
        # Per tile i, [p, k, 0:128] = hi of tokens, [p, k, 128:256] = lo,
        # where h = k*128 + p.
        comb = np.empty((NTILE, 128, KCH, 256), np.float16)
        fh4 = fh.reshape(KCH, 128, NTILE, 128)
        fl4 = fl.reshape(KCH, 128, NTILE, 128)
        comb[:, :, :, 0:128] = fh4.transpose(2, 1, 0, 3)
        comb[:, :, :, 128:256] = fl4.transpose(2, 1, 0, 3)
        # group 2 tiles per DMA: [grp, 128p, t, KCH, 256] -> 16KB/partition line
        comb = np.ascontiguousarray(
            comb.reshape(NTILE // 2, 2, 128, KCH, 256).transpose(0, 2, 1, 3, 4))
        in_maps.append({"hf": comb, "wt": wt, "bias": bias})
    return in_maps


def _run(in_maps, **kwargs):
    nc = _get_nc()
    return run_bass_kernel_spmd(nc, in_maps, core_ids=list(range(NCORES)), **kwargs)


def _assemble(results):
    out = np.empty((NTOK,), np.float32)
    for c in range(NCORES):
        out[c * TPC:(c + 1) * TPC] = results[c]["out"].T.reshape(TPC)
    return out.reshape(B, T)


def kernel(**inputs):
    in_maps = _prep_inputs(**inputs)
    res = _run(in_maps)
    return _assemble(res.results)


def kernel_traced(**inputs):
    """Like kernel() but with NTFF tracing; returns (output, BassKernelResults)."""
    in_maps = _prep_inputs(**inputs)
    res = _run(in_maps, trace=True)
    return _assemble(res.results), res

